# revision 18
# baseline (speedup 1.0000x reference)
"""Multi-head attention + residual + layernorm, sharded over 8 NeuronCores.

Sharding: core c handles batch b = c//4 and query rows [512*(c%4), 512*(c%4+1)).
Each core computes full K/V projections for its batch (replicated across the 4
cores of that batch group), attention for its 512 query rows over all 16 heads,
output projection, residual add and layernorm. Outputs are disjoint row slices
of the final [2, 2048, 1024] tensor, so no cross-core collectives are needed.
"""

import numpy as np
import ml_dtypes

import concourse.bass as bass
import concourse.bacc as bacc
import concourse.tile as tile
import concourse.mybir as mybir
from concourse import bass_utils

F32 = mybir.dt.float32
BF16 = mybir.dt.bfloat16
AF = mybir.ActivationFunctionType
ALU = mybir.AluOpType

B = 2
S = 2048
D = 1024
H = 16
DK = 64
QS = 512          # query rows per core
N_CORES = 8
EPS = 1e-6

NKC = D // 128    # 8 contraction chunks of 128
NKT = S // 128    # 16 key tiles
NQT = QS // 128   # 4 token tiles of the query slice
VW = 66           # per-head stride in v_sb: 64 dims + ones col + pad


def _patch_act_tables():
    """Prefer the table set containing both Exp and Ln so the whole kernel
    uses one ACT table load instead of thrashing between sets."""
    orig = bacc.get_activation_tables
    if getattr(bacc.get_activation_tables, "_attn_patched", False):
        return

    def patched(arch):
        tabs = orig(arch)
        pref = "natural_log_exp_and_others"
        if pref not in tabs:
            return tabs
        out = {}
        for k, v in tabs.items():
            if k == pref:
                out[k] = v
            else:
                out[k] = {f for f in v
                          if str(f).split(".")[-1] not in ("Exp", "Ln")}
        return out

    patched._attn_patched = True
    bacc.get_activation_tables = patched


def _proj_phase(nc, psum_pool, groups, n_chunks, make_mm, finish):
    """Emit a projection phase with the contraction (kc) loop OUTER over
    batches of PSUM groups, so compute starts as soon as the first weight
    chunk lands and the PE stream stays dense."""
    BATCH = 8
    for i in range(0, len(groups), BATCH):
        chunk = groups[i:i + BATCH]
        tiles = []
        for gi, g in enumerate(chunk):
            tiles.append(psum_pool.tile([128, 512], F32, tag="ps",
                                        name=f"ps{i}_{gi}"))
        for kc in range(n_chunks):
            for gi, g in enumerate(chunk):
                make_mm(tiles[gi], g, kc, kc == 0, kc == n_chunks - 1)
        for gi, g in enumerate(chunk):
            finish(tiles[gi], g)


def build_nc():
    _patch_act_tables()
    nc = bacc.Bacc("TRN2", target_bir_lowering=False)

    # DRAM I/O (per-core shapes; host prepares layouts)
    xq_t = nc.dram_tensor("xq_t", [D, QS], BF16, kind="ExternalInput")
    q_nat = nc.dram_tensor("q_nat", [QS, D], F32, kind="ExternalInput")
    xk_t = nc.dram_tensor("xk_t", [D, S], BF16, kind="ExternalInput")
    xv_t = nc.dram_tensor("xv_t", [D, S], BF16, kind="ExternalInput")
    m_t = nc.dram_tensor("m_t", [S, QS], BF16, kind="ExternalInput")
    wq_t = nc.dram_tensor("wq_t", [D, D], BF16, kind="ExternalInput")
    wk_t = nc.dram_tensor("wk_t", [D, D], BF16, kind="ExternalInput")
    wv_t = nc.dram_tensor("wv_t", [D, D], BF16, kind="ExternalInput")
    wo_t = nc.dram_tensor("wo_t", [D, D], BF16, kind="ExternalInput")
    bq_c = nc.dram_tensor("bq_c", [128, NKC], F32, kind="ExternalInput")
    bk_c = nc.dram_tensor("bk_c", [128, NKC], F32, kind="ExternalInput")
    bv_r = nc.dram_tensor("bv_r", [1, D], BF16, kind="ExternalInput")
    bo_r = nc.dram_tensor("bo_r", [1, D], BF16, kind="ExternalInput")
    gamma_r = nc.dram_tensor("gamma_r", [1, D], BF16, kind="ExternalInput")
    beta_r = nc.dram_tensor("beta_r", [1, D], BF16, kind="ExternalInput")
    out = nc.dram_tensor("out", [QS, D], F32, kind="ExternalOutput")

    with tile.TileContext(nc) as tc:
        with (
            tc.tile_pool(name="consts", bufs=1) as consts,
            tc.tile_pool(name="kT", bufs=NKC) as kT_pool,
            tc.tile_pool(name="v", bufs=NKT) as v_pool,
            tc.tile_pool(name="qT", bufs=NKC) as qT_pool,
            tc.tile_pool(name="mT", bufs=NKT) as mT_pool,
            tc.tile_pool(name="qnat", bufs=NQT) as qnat_pool,
            tc.tile_pool(name="oT", bufs=NKC) as oT_pool,
        ):
            # ---- constants / small tensors ----
            ones1_bf = consts.tile([1, 128], BF16)
            nc.vector.memset(ones1_bf, 1.0)
            ones64_f = consts.tile([33, 64], F32)
            nc.vector.memset(ones64_f, 1.0)
            eps_col = consts.tile([128, 1], F32)
            nc.vector.memset(eps_col, EPS)
            bqc_sb = consts.tile([128, NKC], F32)
            nc.sync.dma_start(out=bqc_sb, in_=bq_c[:, :])
            bkc_sb = consts.tile([128, NKC], F32)
            nc.sync.dma_start(out=bkc_sb, in_=bk_c[:, :])
            bvr_sb = consts.tile([1, D], BF16)
            nc.sync.dma_start(out=bvr_sb, in_=bv_r[:, :])
            bor_sb = consts.tile([1, D], BF16)
            nc.sync.dma_start(out=bor_sb, in_=bo_r[:, :])
            gr_sb = consts.tile([1, D], BF16)
            nc.sync.dma_start(out=gr_sb, in_=gamma_r[:, :])
            br_sb = consts.tile([1, D], BF16)
            nc.sync.dma_start(out=br_sb, in_=beta_r[:, :])

            # ---- phase D: V projection -> v_sb (token-major, ones col at 64) ----
            v_sb = [v_pool.tile([128, H, VW], BF16, tag="v", name=f"v{i}")
                    for i in range(NKT)]
            for kt in range(NKT):
                nc.vector.memset(v_sb[kt][:, :, 64:66], 1.0)
            with (
                tc.tile_pool(name="xv", bufs=NKC) as xv_pool,
                tc.tile_pool(name="wv", bufs=NKC) as wv_pool,
                tc.tile_pool(name="psD", bufs=2, space="PSUM") as psD,
            ):
                xv_sb, wv_sb = [], []
                for kc in range(NKC):
                    wt = wv_pool.tile([128, D], BF16, tag="wv", name=f"wv{kc}")
                    nc.sync.dma_start(out=wt, in_=wv_t[kc * 128:(kc + 1) * 128, :])
                    wv_sb.append(wt)
                    xt = xv_pool.tile([128, S], BF16, tag="xv", name=f"xv{kc}")
                    nc.sync.dma_start(out=xt, in_=xv_t[kc * 128:(kc + 1) * 128, :])
                    xv_sb.append(xt)

                def mmD(ps, g, kc, start, stop):
                    kt, nh = g
                    nc.tensor.matmul(ps, xv_sb[kc][:, kt * 128:(kt + 1) * 128],
                                     wv_sb[kc][:, nh * 512:(nh + 1) * 512],
                                     start=start, stop=False)
                    if stop:
                        nc.tensor.matmul(ps, ones1_bf,
                                         bvr_sb[:, nh * 512:(nh + 1) * 512],
                                         start=False, stop=True)

                def finD(ps, g):
                    kt, nh = g
                    nc.vector.tensor_copy(
                        out=v_sb[kt][:, nh * 8:(nh + 1) * 8, 0:64],
                        in_=ps.rearrange("p (a b) -> p a b", a=8))

                groupsD = [(kt, nh) for kt in range(NKT) for nh in range(2)]
                _proj_phase(nc, psD, groupsD, NKC, mmD, finD)

            # gamma/beta broadcast to [128, D] via PE outer product
            gb_sb = consts.tile([128, D], F32)
            bb_sb = consts.tile([128, D], F32)
            with tc.tile_pool(name="gb_ps", bufs=2, space="PSUM") as gb_ps_pool:
                for row_sb, dst in ((gr_sb, gb_sb), (br_sb, bb_sb)):
                    for nh in range(2):
                        ps = gb_ps_pool.tile([128, 512], F32, tag="gbps")
                        nc.tensor.matmul(
                            ps, ones1_bf, row_sb[:, nh * 512:(nh + 1) * 512],
                            start=True, stop=True)
                        nc.vector.tensor_copy(
                            out=dst[:, nh * 512:(nh + 1) * 512], in_=ps)

            # ---- phase B: Q projection -> qT_sb (head-transposed [o, tok]) ----
            qT_sb = [qT_pool.tile([128, QS], BF16, tag="qT", name=f"qT{i}")
                     for i in range(NKC)]
            with (
                tc.tile_pool(name="xq", bufs=NKC) as xq_pool,
                tc.tile_pool(name="wq", bufs=NKC) as wq_pool,
                tc.tile_pool(name="psB", bufs=8, space="PSUM") as psB,
            ):
                xq_sb, wq_sb = [], []
                for kc in range(NKC):
                    wt = wq_pool.tile([128, D], BF16, tag="wq", name=f"wq{kc}")
                    nc.sync.dma_start(out=wt, in_=wq_t[kc * 128:(kc + 1) * 128, :])
                    wq_sb.append(wt)
                    xt = xq_pool.tile([128, QS], BF16, tag="xq", name=f"xq{kc}")
                    nc.sync.dma_start(out=xt, in_=xq_t[kc * 128:(kc + 1) * 128, :])
                    xq_sb.append(xt)

                def mmB(ps, ot, kc, start, stop):
                    nc.tensor.matmul(ps, wq_sb[kc][:, ot * 128:(ot + 1) * 128],
                                     xq_sb[kc][:, 0:QS], start=start, stop=stop)

                def finB(ps, ot):
                    nc.vector.tensor_scalar(
                        out=qT_sb[ot], in0=ps, scalar1=bqc_sb[:, ot:ot + 1],
                        scalar2=None, op0=ALU.add)

                _proj_phase(nc, psB, list(range(NKC)), NKC, mmB, finB)

            # ---- phase C: K projection -> kT_sb ----
            kT_sb = [kT_pool.tile([128, S], BF16, tag="kT", name=f"kT{i}")
                     for i in range(NKC)]
            with (
                tc.tile_pool(name="xk", bufs=NKC) as xk_pool,
                tc.tile_pool(name="wk", bufs=NKC) as wk_pool,
                tc.tile_pool(name="psC", bufs=8, space="PSUM") as psC,
            ):
                xk_sb, wk_sb = [], []
                for kc in range(NKC):
                    wt = wk_pool.tile([128, D], BF16, tag="wk", name=f"wk{kc}")
                    nc.sync.dma_start(out=wt, in_=wk_t[kc * 128:(kc + 1) * 128, :])
                    wk_sb.append(wt)
                    xt = xk_pool.tile([128, S], BF16, tag="xk", name=f"xk{kc}")
                    nc.sync.dma_start(out=xt, in_=xk_t[kc * 128:(kc + 1) * 128, :])
                    xk_sb.append(xt)

                def mmC(ps, g, kc, start, stop):
                    ot, tck = g
                    nc.tensor.matmul(ps, wk_sb[kc][:, ot * 128:(ot + 1) * 128],
                                     xk_sb[kc][:, tck * 512:(tck + 1) * 512],
                                     start=start, stop=stop)

                def finC(ps, g):
                    ot, tck = g
                    nc.vector.tensor_scalar(
                        out=kT_sb[ot][:, tck * 512:(tck + 1) * 512],
                        in0=ps, scalar1=bkc_sb[:, ot:ot + 1], scalar2=None,
                        op0=ALU.add)

                groupsC = [(ot, tck) for ot in range(NKC) for tck in range(S // 512)]
                _proj_phase(nc, psC, groupsC, NKC, mmC, finC)

            # mask tiles + residual + wo prefetch (needed by E/F)
            mT_sb = []
            for kt in range(NKT):
                t = mT_pool.tile([128, QS], BF16, tag="mT", name=f"mT{kt}")
                nc.sync.dma_start(out=t, in_=m_t[kt * 128:(kt + 1) * 128, :])
                mT_sb.append(t)
            qnat_sb = []
            for tt in range(NQT):
                t = qnat_pool.tile([128, D], F32, tag="qnat", name=f"qnat{tt}")
                nc.sync.dma_start(out=t, in_=q_nat[tt * 128:(tt + 1) * 128, :])
                qnat_sb.append(t)

            # ---- phase E: attention (head pairs; scores transposed [k, q]) ----
            oT_sb = [oT_pool.tile([128, QS], BF16, tag="oT", name=f"oT{i}")
                     for i in range(NKC)]
            with (
                tc.tile_pool(name="psS", bufs=2, space="PSUM") as psS,
                tc.tile_pool(name="psO", bufs=2, space="PSUM") as psO,
                tc.tile_pool(name="pp", bufs=8) as pp_pool,
                tc.tile_pool(name="att_sm", bufs=6) as att_sm,
            ):
                for j in range(H // 2):
                    o_ps = [psO.tile([65, QS], F32, tag="psO", name=f"ops{i}")
                            for i in range(2)]
                    for kt in range(NKT):
                        s_ps = psS.tile([128, 2 * QS], F32, tag="psS")
                        for hi in range(2):
                            nc.tensor.matmul(
                                s_ps[:, hi * QS:(hi + 1) * QS],
                                kT_sb[j][hi * 64:(hi + 1) * 64,
                                         kt * 128:(kt + 1) * 128],
                                qT_sb[j][hi * 64:(hi + 1) * 64, 0:QS],
                                start=True, stop=True,
                                tile_position=(hi * 64, 0),
                            )
                        p_sb = pp_pool.tile([128, 2 * QS], BF16, tag="pp")
                        nc.scalar.activation(out=p_sb, in_=s_ps, func=AF.Exp)
                        m_rep = bass.AP(
                            tensor=mT_sb[kt].tensor, offset=mT_sb[kt].offset,
                            ap=[mT_sb[kt].ap[0], [0, 2], [1, QS]],
                        )
                        nc.vector.tensor_tensor(
                            out=p_sb.rearrange("p (a b) -> p a b", a=2),
                            in0=p_sb.rearrange("p (a b) -> p a b", a=2),
                            in1=m_rep, op=ALU.mult,
                        )
                        for hi in range(2):
                            nc.tensor.matmul(
                                o_ps[hi],
                                v_sb[kt][:, 2 * j + hi, 0:65],
                                p_sb[:, hi * QS:(hi + 1) * QS],
                                start=(kt == 0), stop=(kt == NKT - 1),
                            )
                    # r = exp(-ln(d)); one [1,QS] row per head, then
                    # partition-broadcast on the idle GPSIMD engine
                    for hi in range(2):
                        r_row = att_sm.tile([1, QS], F32, tag=f"rrow{hi}",
                                            name=f"rrow{hi}")
                        nc.scalar.activation(out=r_row,
                                             in_=o_ps[hi][64:65, :],
                                             func=AF.Ln)
                        nc.scalar.activation(out=r_row, in_=r_row,
                                             func=AF.Exp, scale=-1.0)
                        r_bc = att_sm.tile([64, QS], F32, tag="rbc")
                        nc.gpsimd.partition_broadcast(r_bc, r_row)
                        nc.vector.tensor_tensor(
                            out=oT_sb[j][hi * 64:(hi + 1) * 64, :],
                            in0=o_ps[hi][0:64, :], in1=r_bc, op=ALU.mult,
                        )

            # ---- phase F: output projection + residual ----
            with (
                tc.tile_pool(name="wo", bufs=NKC) as wo_pool,
                tc.tile_pool(name="psF", bufs=4, space="PSUM") as psF,
            ):
                wo_sb = []
                for kc in range(NKC):
                    wt = wo_pool.tile([128, D], BF16, tag="wo", name=f"wo{kc}")
                    nc.sync.dma_start(out=wt, in_=wo_t[kc * 128:(kc + 1) * 128, :])
                    wo_sb.append(wt)
                for tt in range(NQT):
                    for nh in range(2):
                        ps = psF.tile([128, 512], F32, tag="psF")
                        for jc in range(NKC):
                            nc.tensor.matmul(
                                ps, oT_sb[jc][:, tt * 128:(tt + 1) * 128],
                                wo_sb[jc][:, nh * 512:(nh + 1) * 512],
                                start=(jc == 0), stop=False)
                        nc.tensor.matmul(
                            ps, ones1_bf, bor_sb[:, nh * 512:(nh + 1) * 512],
                            start=False, stop=True)
                        nc.vector.tensor_tensor(
                            out=qnat_sb[tt][:, nh * 512:(nh + 1) * 512],
                            in0=ps, in1=qnat_sb[tt][:, nh * 512:(nh + 1) * 512],
                            op=ALU.add)

            # ---- phase G: layernorm + store ----
            with tc.tile_pool(name="ln", bufs=2 * NQT) as ln_pool:
                for tt in range(NQT):
                    y = qnat_sb[tt]
                    stats = ln_pool.tile([128, 2, 6], F32, tag="stats")
                    for half in range(2):
                        nc.vector.bn_stats(
                            out=stats[:, half, :],
                            in_=y[:, half * 512:(half + 1) * 512])
                    mv = ln_pool.tile([128, 2], F32, tag="mv")
                    nc.vector.bn_aggr(out=mv, in_=stats)
                    rstd = ln_pool.tile([128, 1], F32, tag="rstd")
                    nc.scalar.activation(out=rstd, in_=mv[:, 1:2], func=AF.Ln,
                                         bias=eps_col, scale=1.0)
                    nc.scalar.activation(out=rstd, in_=rstd, func=AF.Exp,
                                         scale=-0.5)
                    nc.vector.tensor_scalar(
                        out=y, in0=y, scalar1=mv[:, 0:1], scalar2=rstd,
                        op0=ALU.subtract, op1=ALU.mult)
                    nc.vector.tensor_tensor(out=y, in0=y, in1=gb_sb, op=ALU.mult)
                    nc.vector.tensor_tensor(out=y, in0=y, in1=bb_sb, op=ALU.add)
                    nc.sync.dma_start(out=out[tt * 128:(tt + 1) * 128, :], in_=y)

    nc.compile()
    return nc


_NC_CACHE = None


def _get_nc():
    global _NC_CACHE
    if _NC_CACHE is None:
        _NC_CACHE = build_nc()
    return _NC_CACHE


def _prep_in_maps(query, key, values, mask, Wq, bq, Wk, bk, Wv, bv, Wo, bo,
                  gamma, beta):
    bf = ml_dtypes.bfloat16
    f32 = np.float32

    def c(x):
        return np.ascontiguousarray(x)

    wq_t = c((Wq.T / 8.0).astype(bf))
    wk_t = c(Wk.T.astype(bf))
    wv_t = c(Wv.T.astype(bf))
    wo_t = c(Wo.T.astype(bf))
    bq_c = c((bq.astype(f32) / 8.0).reshape(NKC, 128).T)
    bk_c = c(bk.astype(f32).reshape(NKC, 128).T)
    bv_r = c(bv.astype(bf)[None, :])
    bo_r = c(bo.astype(bf)[None, :])
    gamma_r = c(gamma.astype(bf)[None, :])
    beta_r = c(beta.astype(bf)[None, :])

    xk_t = [c(key[b].T.astype(bf)) for b in range(B)]
    xv_t = [c(values[b].T.astype(bf)) for b in range(B)]
    m_tb = [c(mask[b].T.astype(bf)) for b in range(B)]
    xq_tb = [c(query[b].T.astype(bf)) for b in range(B)]

    in_maps = []
    for core in range(N_CORES):
        b = core // 4
        qs = core % 4
        in_maps.append({
            "xq_t": c(xq_tb[b][:, qs * QS:(qs + 1) * QS]),
            "q_nat": c(query[b, qs * QS:(qs + 1) * QS, :].astype(f32)),
            "xk_t": xk_t[b],
            "xv_t": xv_t[b],
            "m_t": c(m_tb[b][:, qs * QS:(qs + 1) * QS]),
            "wq_t": wq_t, "wk_t": wk_t, "wv_t": wv_t, "wo_t": wo_t,
            "bq_c": bq_c, "bk_c": bk_c, "bv_r": bv_r, "bo_r": bo_r,
            "gamma_r": gamma_r, "beta_r": beta_r,
        })
    return in_maps


def kernel(query, key, values, mask, Wq, bq, Wk, bk, Wv, bv, Wo, bo, gamma,
           beta, _trace=False):
    query = np.asarray(query, dtype=np.float32)
    key = np.asarray(key, dtype=np.float32)
    values = np.asarray(values, dtype=np.float32)
    mask = np.asarray(mask)
    in_maps = _prep_in_maps(query, key, values, mask,
                            np.asarray(Wq, np.float32), np.asarray(bq, np.float32),
                            np.asarray(Wk, np.float32), np.asarray(bk, np.float32),
                            np.asarray(Wv, np.float32), np.asarray(bv, np.float32),
                            np.asarray(Wo, np.float32), np.asarray(bo, np.float32),
                            np.asarray(gamma, np.float32), np.asarray(beta, np.float32))
    nc = _get_nc()
    res = bass_utils.run_bass_kernel_spmd(
        nc, in_maps, core_ids=list(range(N_CORES)), trace=_trace,
    )
    outp = np.empty((B, S, D), dtype=np.float32)
    for core in range(N_CORES):
        b = core // 4
        qs = core % 4
        outp[b, qs * QS:(qs + 1) * QS, :] = res.results[core]["out"]
    if _trace:
        kernel._last_results = res
    return outp


# revision 19
# speedup vs baseline: 1.0445x; 1.0445x over previous
"""Multi-head attention + residual + layernorm, sharded over 8 NeuronCores.

Sharding: core c handles batch b = c//4 and query rows [512*(c%4), 512*(c%4+1)).
Each core computes full K/V projections for its batch (replicated across the 4
cores of that batch group), attention for its 512 query rows over all 16 heads,
output projection, residual add and layernorm. Outputs are disjoint row slices
of the final [2, 2048, 1024] tensor, so no cross-core collectives are needed.
"""

import numpy as np
import ml_dtypes

import concourse.bass as bass
import concourse.bacc as bacc
import concourse.tile as tile
import concourse.mybir as mybir
from concourse import bass_utils

F32 = mybir.dt.float32
BF16 = mybir.dt.bfloat16
AF = mybir.ActivationFunctionType
ALU = mybir.AluOpType

B = 2
S = 2048
D = 1024
H = 16
DK = 64
QS = 512          # query rows per core
N_CORES = 8
EPS = 1e-6

NKC = D // 128    # 8 contraction chunks of 128
NKT = S // 128    # 16 key tiles
NQT = QS // 128   # 4 token tiles of the query slice
VW = 66           # per-head stride in v_sb: 64 dims + ones col + pad


def _patch_act_tables():
    """Prefer the table set containing both Exp and Ln so the whole kernel
    uses one ACT table load instead of thrashing between sets."""
    orig = bacc.get_activation_tables
    if getattr(bacc.get_activation_tables, "_attn_patched", False):
        return

    def patched(arch):
        tabs = orig(arch)
        pref = "natural_log_exp_and_others"
        if pref not in tabs:
            return tabs
        out = {}
        for k, v in tabs.items():
            if k == pref:
                out[k] = v
            else:
                out[k] = {f for f in v
                          if str(f).split(".")[-1] not in ("Exp", "Ln")}
        return out

    patched._attn_patched = True
    bacc.get_activation_tables = patched


def _proj_phase(nc, psum_pool, groups, n_chunks, make_mm, finish):
    """Emit a projection phase with the contraction (kc) loop OUTER over
    batches of PSUM groups, so compute starts as soon as the first weight
    chunk lands and the PE stream stays dense."""
    BATCH = 8
    for i in range(0, len(groups), BATCH):
        chunk = groups[i:i + BATCH]
        tiles = []
        for gi, g in enumerate(chunk):
            tiles.append(psum_pool.tile([128, 512], F32, tag="ps",
                                        name=f"ps{i}_{gi}"))
        for kc in range(n_chunks):
            for gi, g in enumerate(chunk):
                make_mm(tiles[gi], g, kc, kc == 0, kc == n_chunks - 1)
        for gi, g in enumerate(chunk):
            finish(tiles[gi], g)


def build_nc():
    _patch_act_tables()
    nc = bacc.Bacc("TRN2", target_bir_lowering=False)

    # DRAM I/O (per-core shapes; host prepares layouts)
    xq_t = nc.dram_tensor("xq_t", [D, QS], BF16, kind="ExternalInput")
    q_nat = nc.dram_tensor("q_nat", [QS, D], F32, kind="ExternalInput")
    xk_t = nc.dram_tensor("xk_t", [D, S], BF16, kind="ExternalInput")
    xv_t = nc.dram_tensor("xv_t", [D, S], BF16, kind="ExternalInput")
    m_t = nc.dram_tensor("m_t", [S, QS], BF16, kind="ExternalInput")
    wq_t = nc.dram_tensor("wq_t", [D, D], BF16, kind="ExternalInput")
    wk_t = nc.dram_tensor("wk_t", [D, D], BF16, kind="ExternalInput")
    wv_t = nc.dram_tensor("wv_t", [D, D], BF16, kind="ExternalInput")
    wo_t = nc.dram_tensor("wo_t", [D, D], BF16, kind="ExternalInput")
    bq_c = nc.dram_tensor("bq_c", [128, NKC], F32, kind="ExternalInput")
    bk_c = nc.dram_tensor("bk_c", [128, NKC], F32, kind="ExternalInput")
    bv_r = nc.dram_tensor("bv_r", [1, D], BF16, kind="ExternalInput")
    bo_r = nc.dram_tensor("bo_r", [1, D], BF16, kind="ExternalInput")
    gamma_r = nc.dram_tensor("gamma_r", [1, D], BF16, kind="ExternalInput")
    beta_r = nc.dram_tensor("beta_r", [1, D], BF16, kind="ExternalInput")
    out = nc.dram_tensor("out", [QS, D], F32, kind="ExternalOutput")

    with tile.TileContext(nc) as tc:
        with (
            tc.tile_pool(name="consts", bufs=1) as consts,
            tc.tile_pool(name="kT", bufs=NKC) as kT_pool,
            tc.tile_pool(name="v", bufs=NKT) as v_pool,
            tc.tile_pool(name="qT", bufs=NKC) as qT_pool,
            tc.tile_pool(name="mT", bufs=NKT) as mT_pool,
            tc.tile_pool(name="qnat", bufs=NQT) as qnat_pool,
            tc.tile_pool(name="oT", bufs=NKC) as oT_pool,
        ):
            # ---- constants / small tensors ----
            ones1_bf = consts.tile([1, 128], BF16)
            nc.vector.memset(ones1_bf, 1.0)
            ones64_f = consts.tile([33, 64], F32)
            nc.vector.memset(ones64_f, 1.0)
            eps_col = consts.tile([128, 1], F32)
            nc.vector.memset(eps_col, EPS)
            bqc_sb = consts.tile([128, NKC], F32)
            nc.sync.dma_start(out=bqc_sb, in_=bq_c[:, :])
            bkc_sb = consts.tile([128, NKC], F32)
            nc.sync.dma_start(out=bkc_sb, in_=bk_c[:, :])
            bvr_sb = consts.tile([1, D], BF16)
            nc.sync.dma_start(out=bvr_sb, in_=bv_r[:, :])
            bor_sb = consts.tile([1, D], BF16)
            nc.sync.dma_start(out=bor_sb, in_=bo_r[:, :])
            gr_sb = consts.tile([1, D], BF16)
            nc.sync.dma_start(out=gr_sb, in_=gamma_r[:, :])
            br_sb = consts.tile([1, D], BF16)
            nc.sync.dma_start(out=br_sb, in_=beta_r[:, :])

            # gamma/beta broadcast to [128, D] via PE outer product
            gb_sb = consts.tile([128, D], F32)
            bb_sb = consts.tile([128, D], F32)
            with tc.tile_pool(name="gb_ps", bufs=2, space="PSUM") as gb_ps_pool:
                for row_sb, dst in ((gr_sb, gb_sb), (br_sb, bb_sb)):
                    for nh in range(2):
                        ps = gb_ps_pool.tile([128, 512], F32, tag="gbps")
                        nc.tensor.matmul(
                            ps, ones1_bf, row_sb[:, nh * 512:(nh + 1) * 512],
                            start=True, stop=True)
                        nc.vector.tensor_copy(
                            out=dst[:, nh * 512:(nh + 1) * 512], in_=ps)

            # ---- phase B: Q projection -> qT_sb (head-transposed [o, tok]) ----
            qT_sb = [qT_pool.tile([128, QS], BF16, tag="qT", name=f"qT{i}")
                     for i in range(NKC)]
            with (
                tc.tile_pool(name="xq", bufs=NKC) as xq_pool,
                tc.tile_pool(name="wq", bufs=NKC) as wq_pool,
                tc.tile_pool(name="psB", bufs=8, space="PSUM") as psB,
            ):
                xq_sb, wq_sb = [], []
                for kc in range(NKC):
                    wt = wq_pool.tile([128, D], BF16, tag="wq", name=f"wq{kc}")
                    nc.sync.dma_start(out=wt, in_=wq_t[kc * 128:(kc + 1) * 128, :])
                    wq_sb.append(wt)
                    xt = xq_pool.tile([128, QS], BF16, tag="xq", name=f"xq{kc}")
                    nc.sync.dma_start(out=xt, in_=xq_t[kc * 128:(kc + 1) * 128, :])
                    xq_sb.append(xt)

                def mmB(ps, ot, kc, start, stop):
                    nc.tensor.matmul(ps, wq_sb[kc][:, ot * 128:(ot + 1) * 128],
                                     xq_sb[kc][:, 0:QS], start=start, stop=stop)

                def finB(ps, ot):
                    nc.vector.tensor_scalar(
                        out=qT_sb[ot], in0=ps, scalar1=bqc_sb[:, ot:ot + 1],
                        scalar2=None, op0=ALU.add)

                _proj_phase(nc, psB, list(range(NKC)), NKC, mmB, finB)

            # ---- phase C: K projection -> kT_sb ----
            kT_sb = [kT_pool.tile([128, S], BF16, tag="kT", name=f"kT{i}")
                     for i in range(NKC)]
            with (
                tc.tile_pool(name="xk", bufs=NKC) as xk_pool,
                tc.tile_pool(name="wk", bufs=NKC) as wk_pool,
                tc.tile_pool(name="psC", bufs=8, space="PSUM") as psC,
            ):
                xk_sb, wk_sb = [], []
                for kc in range(NKC):
                    wt = wk_pool.tile([128, D], BF16, tag="wk", name=f"wk{kc}")
                    nc.sync.dma_start(out=wt, in_=wk_t[kc * 128:(kc + 1) * 128, :])
                    wk_sb.append(wt)
                    xt = xk_pool.tile([128, S], BF16, tag="xk", name=f"xk{kc}")
                    nc.sync.dma_start(out=xt, in_=xk_t[kc * 128:(kc + 1) * 128, :])
                    xk_sb.append(xt)

                def mmC(ps, g, kc, start, stop):
                    ot, tck = g
                    nc.tensor.matmul(ps, wk_sb[kc][:, ot * 128:(ot + 1) * 128],
                                     xk_sb[kc][:, tck * 512:(tck + 1) * 512],
                                     start=start, stop=stop)

                def finC(ps, g):
                    ot, tck = g
                    nc.vector.tensor_scalar(
                        out=kT_sb[ot][:, tck * 512:(tck + 1) * 512],
                        in0=ps, scalar1=bkc_sb[:, ot:ot + 1], scalar2=None,
                        op0=ALU.add)

                groupsC = [(ot, tck) for ot in range(NKC) for tck in range(S // 512)]
                _proj_phase(nc, psC, groupsC, NKC, mmC, finC)

            # ---- phase D: V projection -> v_sb (token-major, ones col at 64) ----
            v_sb = [v_pool.tile([128, H, VW], BF16, tag="v", name=f"v{i}")
                    for i in range(NKT)]
            for kt in range(NKT):
                nc.vector.memset(v_sb[kt][:, :, 64:66], 1.0)
            with (
                tc.tile_pool(name="xv", bufs=NKC) as xv_pool,
                tc.tile_pool(name="wv", bufs=NKC) as wv_pool,
                tc.tile_pool(name="psD", bufs=8, space="PSUM") as psD,
            ):
                xv_sb, wv_sb = [], []
                for kc in range(NKC):
                    wt = wv_pool.tile([128, D], BF16, tag="wv", name=f"wv{kc}")
                    nc.sync.dma_start(out=wt, in_=wv_t[kc * 128:(kc + 1) * 128, :])
                    wv_sb.append(wt)
                    xt = xv_pool.tile([128, S], BF16, tag="xv", name=f"xv{kc}")
                    nc.sync.dma_start(out=xt, in_=xv_t[kc * 128:(kc + 1) * 128, :])
                    xv_sb.append(xt)

                def mmD(ps, g, kc, start, stop):
                    kt, nh = g
                    nc.tensor.matmul(ps, xv_sb[kc][:, kt * 128:(kt + 1) * 128],
                                     wv_sb[kc][:, nh * 512:(nh + 1) * 512],
                                     start=start, stop=False)
                    if stop:
                        nc.tensor.matmul(ps, ones1_bf,
                                         bvr_sb[:, nh * 512:(nh + 1) * 512],
                                         start=False, stop=True)

                def finD(ps, g):
                    kt, nh = g
                    nc.vector.tensor_copy(
                        out=v_sb[kt][:, nh * 8:(nh + 1) * 8, 0:64],
                        in_=ps.rearrange("p (a b) -> p a b", a=8))

                groupsD = [(kt, nh) for kt in range(NKT) for nh in range(2)]
                _proj_phase(nc, psD, groupsD, NKC, mmD, finD)

            # mask tiles + residual + wo prefetch (needed by E/F)
            mT_sb = []
            for kt in range(NKT):
                t = mT_pool.tile([128, QS], BF16, tag="mT", name=f"mT{kt}")
                nc.sync.dma_start(out=t, in_=m_t[kt * 128:(kt + 1) * 128, :])
                mT_sb.append(t)
            qnat_sb = []
            for tt in range(NQT):
                t = qnat_pool.tile([128, D], F32, tag="qnat", name=f"qnat{tt}")
                nc.sync.dma_start(out=t, in_=q_nat[tt * 128:(tt + 1) * 128, :])
                qnat_sb.append(t)

            # ---- phase E: attention (head pairs; scores transposed [k, q]) ----
            oT_sb = [oT_pool.tile([128, QS], BF16, tag="oT", name=f"oT{i}")
                     for i in range(NKC)]
            with (
                tc.tile_pool(name="psS", bufs=2, space="PSUM") as psS,
                tc.tile_pool(name="psO", bufs=4, space="PSUM") as psO,
                tc.tile_pool(name="pp", bufs=8) as pp_pool,
                tc.tile_pool(name="att_sm", bufs=6) as att_sm,
            ):
                for jj in range(0, H // 2, 2):
                    o_ps = {}
                    for j in (jj, jj + 1):
                        o_ps[j] = [psO.tile([65, QS], F32, tag="psO",
                                            name=f"ops{j}_{i}") for i in range(2)]
                    for kt in range(NKT):
                        for j in (jj, jj + 1):
                            s_ps = psS.tile([128, 2 * QS], F32, tag="psS")
                            for hi in range(2):
                                nc.tensor.matmul(
                                    s_ps[:, hi * QS:(hi + 1) * QS],
                                    kT_sb[j][hi * 64:(hi + 1) * 64,
                                             kt * 128:(kt + 1) * 128],
                                    qT_sb[j][hi * 64:(hi + 1) * 64, 0:QS],
                                    start=True, stop=True,
                                    tile_position=(hi * 64, 0),
                                )
                            p_sb = pp_pool.tile([128, 2 * QS], BF16, tag="pp")
                            nc.scalar.activation(out=p_sb, in_=s_ps, func=AF.Exp)
                            m_rep = bass.AP(
                                tensor=mT_sb[kt].tensor, offset=mT_sb[kt].offset,
                                ap=[mT_sb[kt].ap[0], [0, 2], [1, QS]],
                            )
                            nc.vector.tensor_tensor(
                                out=p_sb.rearrange("p (a b) -> p a b", a=2),
                                in0=p_sb.rearrange("p (a b) -> p a b", a=2),
                                in1=m_rep, op=ALU.mult,
                            )
                            for hi in range(2):
                                nc.tensor.matmul(
                                    o_ps[j][hi],
                                    v_sb[kt][:, 2 * j + hi, 0:65],
                                    p_sb[:, hi * QS:(hi + 1) * QS],
                                    start=(kt == 0), stop=(kt == NKT - 1),
                                )
                    for j in (jj, jj + 1):
                        # r = exp(-ln(d)); one [1,QS] row per head, then
                        # partition-broadcast on the idle GPSIMD engine
                        for hi in range(2):
                            r_row = att_sm.tile([1, QS], F32, tag=f"rrow{hi}",
                                                name=f"rrow{hi}")
                            nc.scalar.activation(out=r_row,
                                                 in_=o_ps[j][hi][64:65, :],
                                                 func=AF.Ln)
                            nc.scalar.activation(out=r_row, in_=r_row,
                                                 func=AF.Exp, scale=-1.0)
                            r_bc = att_sm.tile([64, QS], F32, tag="rbc")
                            nc.gpsimd.partition_broadcast(r_bc, r_row)
                            nc.vector.tensor_tensor(
                                out=oT_sb[j][hi * 64:(hi + 1) * 64, :],
                                in0=o_ps[j][hi][0:64, :], in1=r_bc, op=ALU.mult,
                            )

            # ---- phase F: output projection + residual ----
            with (
                tc.tile_pool(name="wo", bufs=NKC) as wo_pool,
                tc.tile_pool(name="psF", bufs=4, space="PSUM") as psF,
            ):
                wo_sb = []
                for kc in range(NKC):
                    wt = wo_pool.tile([128, D], BF16, tag="wo", name=f"wo{kc}")
                    nc.sync.dma_start(out=wt, in_=wo_t[kc * 128:(kc + 1) * 128, :])
                    wo_sb.append(wt)
                for tt in range(NQT):
                    for nh in range(2):
                        ps = psF.tile([128, 512], F32, tag="psF")
                        for jc in range(NKC):
                            nc.tensor.matmul(
                                ps, oT_sb[jc][:, tt * 128:(tt + 1) * 128],
                                wo_sb[jc][:, nh * 512:(nh + 1) * 512],
                                start=(jc == 0), stop=False)
                        nc.tensor.matmul(
                            ps, ones1_bf, bor_sb[:, nh * 512:(nh + 1) * 512],
                            start=False, stop=True)
                        nc.vector.tensor_tensor(
                            out=qnat_sb[tt][:, nh * 512:(nh + 1) * 512],
                            in0=ps, in1=qnat_sb[tt][:, nh * 512:(nh + 1) * 512],
                            op=ALU.add)

            # ---- phase G: layernorm + store ----
            with tc.tile_pool(name="ln", bufs=2 * NQT) as ln_pool:
                for tt in range(NQT):
                    y = qnat_sb[tt]
                    stats = ln_pool.tile([128, 2, 6], F32, tag="stats")
                    for half in range(2):
                        nc.vector.bn_stats(
                            out=stats[:, half, :],
                            in_=y[:, half * 512:(half + 1) * 512])
                    mv = ln_pool.tile([128, 2], F32, tag="mv")
                    nc.vector.bn_aggr(out=mv, in_=stats)
                    rstd = ln_pool.tile([128, 1], F32, tag="rstd")
                    nc.scalar.activation(out=rstd, in_=mv[:, 1:2], func=AF.Ln,
                                         bias=eps_col, scale=1.0)
                    nc.scalar.activation(out=rstd, in_=rstd, func=AF.Exp,
                                         scale=-0.5)
                    nc.vector.tensor_scalar(
                        out=y, in0=y, scalar1=mv[:, 0:1], scalar2=rstd,
                        op0=ALU.subtract, op1=ALU.mult)
                    nc.vector.tensor_tensor(out=y, in0=y, in1=gb_sb, op=ALU.mult)
                    nc.vector.tensor_tensor(out=y, in0=y, in1=bb_sb, op=ALU.add)
                    nc.sync.dma_start(out=out[tt * 128:(tt + 1) * 128, :], in_=y)

    nc.compile()
    return nc


_NC_CACHE = None


def _get_nc():
    global _NC_CACHE
    if _NC_CACHE is None:
        _NC_CACHE = build_nc()
    return _NC_CACHE


def _prep_in_maps(query, key, values, mask, Wq, bq, Wk, bk, Wv, bv, Wo, bo,
                  gamma, beta):
    bf = ml_dtypes.bfloat16
    f32 = np.float32

    def c(x):
        return np.ascontiguousarray(x)

    wq_t = c((Wq.T / 8.0).astype(bf))
    wk_t = c(Wk.T.astype(bf))
    wv_t = c(Wv.T.astype(bf))
    wo_t = c(Wo.T.astype(bf))
    bq_c = c((bq.astype(f32) / 8.0).reshape(NKC, 128).T)
    bk_c = c(bk.astype(f32).reshape(NKC, 128).T)
    bv_r = c(bv.astype(bf)[None, :])
    bo_r = c(bo.astype(bf)[None, :])
    gamma_r = c(gamma.astype(bf)[None, :])
    beta_r = c(beta.astype(bf)[None, :])

    xk_t = [c(key[b].T.astype(bf)) for b in range(B)]
    xv_t = [c(values[b].T.astype(bf)) for b in range(B)]
    m_tb = [c(mask[b].T.astype(bf)) for b in range(B)]
    xq_tb = [c(query[b].T.astype(bf)) for b in range(B)]

    in_maps = []
    for core in range(N_CORES):
        b = core // 4
        qs = core % 4
        in_maps.append({
            "xq_t": c(xq_tb[b][:, qs * QS:(qs + 1) * QS]),
            "q_nat": c(query[b, qs * QS:(qs + 1) * QS, :].astype(f32)),
            "xk_t": xk_t[b],
            "xv_t": xv_t[b],
            "m_t": c(m_tb[b][:, qs * QS:(qs + 1) * QS]),
            "wq_t": wq_t, "wk_t": wk_t, "wv_t": wv_t, "wo_t": wo_t,
            "bq_c": bq_c, "bk_c": bk_c, "bv_r": bv_r, "bo_r": bo_r,
            "gamma_r": gamma_r, "beta_r": beta_r,
        })
    return in_maps


def kernel(query, key, values, mask, Wq, bq, Wk, bk, Wv, bv, Wo, bo, gamma,
           beta, _trace=False):
    query = np.asarray(query, dtype=np.float32)
    key = np.asarray(key, dtype=np.float32)
    values = np.asarray(values, dtype=np.float32)
    mask = np.asarray(mask)
    in_maps = _prep_in_maps(query, key, values, mask,
                            np.asarray(Wq, np.float32), np.asarray(bq, np.float32),
                            np.asarray(Wk, np.float32), np.asarray(bk, np.float32),
                            np.asarray(Wv, np.float32), np.asarray(bv, np.float32),
                            np.asarray(Wo, np.float32), np.asarray(bo, np.float32),
                            np.asarray(gamma, np.float32), np.asarray(beta, np.float32))
    nc = _get_nc()
    res = bass_utils.run_bass_kernel_spmd(
        nc, in_maps, core_ids=list(range(N_CORES)), trace=_trace,
    )
    outp = np.empty((B, S, D), dtype=np.float32)
    for core in range(N_CORES):
        b = core // 4
        qs = core % 4
        outp[b, qs * QS:(qs + 1) * QS, :] = res.results[core]["out"]
    if _trace:
        kernel._last_results = res
    return outp


# revision 20
# speedup vs baseline: 1.1080x; 1.0609x over previous
"""Multi-head attention + residual + layernorm, sharded over 8 NeuronCores.

Sharding: core c handles batch b = c//4 and query rows [512*(c%4), 512*(c%4+1)).
Each core computes full K/V projections for its batch (replicated across the 4
cores of that batch group), attention for its 512 query rows over all 16 heads,
output projection, residual add and layernorm. Outputs are disjoint row slices
of the final [2, 2048, 1024] tensor, so no cross-core collectives are needed.
"""

import numpy as np
import ml_dtypes

import concourse.bass as bass
import concourse.bacc as bacc
import concourse.tile as tile
import concourse.mybir as mybir
from concourse import bass_utils

F32 = mybir.dt.float32
BF16 = mybir.dt.bfloat16
AF = mybir.ActivationFunctionType
ALU = mybir.AluOpType

B = 2
S = 2048
D = 1024
H = 16
DK = 64
QS = 512          # query rows per core
N_CORES = 8
EPS = 1e-6

NKC = D // 128    # 8 contraction chunks of 128
NKT = S // 128    # 16 key tiles
NQT = QS // 128   # 4 token tiles of the query slice
VW = 66           # per-head stride in v_sb: 64 dims + ones col + pad


def _patch_act_tables():
    """Prefer the table set containing both Exp and Ln so the whole kernel
    uses one ACT table load instead of thrashing between sets."""
    orig = bacc.get_activation_tables
    if getattr(bacc.get_activation_tables, "_attn_patched", False):
        return

    def patched(arch):
        tabs = orig(arch)
        pref = "natural_log_exp_and_others"
        if pref not in tabs:
            return tabs
        out = {}
        for k, v in tabs.items():
            if k == pref:
                out[k] = v
            else:
                out[k] = {f for f in v
                          if str(f).split(".")[-1] not in ("Exp", "Ln")}
        return out

    patched._attn_patched = True
    bacc.get_activation_tables = patched


def _proj_phase(nc, psum_pool, groups, n_chunks, make_mm, finish):
    """Emit a projection phase with the contraction (kc) loop OUTER over
    batches of PSUM groups, so compute starts as soon as the first weight
    chunk lands and the PE stream stays dense."""
    BATCH = 8
    for i in range(0, len(groups), BATCH):
        chunk = groups[i:i + BATCH]
        tiles = []
        for gi, g in enumerate(chunk):
            tiles.append(psum_pool.tile([128, 512], F32, tag="ps",
                                        name=f"ps{i}_{gi}"))
        for kc in range(n_chunks):
            for gi, g in enumerate(chunk):
                make_mm(tiles[gi], g, kc, kc == 0, kc == n_chunks - 1)
        for gi, g in enumerate(chunk):
            finish(tiles[gi], g)


def build_nc():
    _patch_act_tables()
    nc = bacc.Bacc("TRN2", target_bir_lowering=False)

    # DRAM I/O (per-core shapes; host prepares layouts)
    xq_t = nc.dram_tensor("xq_t", [D, QS], BF16, kind="ExternalInput")
    q_nat = nc.dram_tensor("q_nat", [QS, D], F32, kind="ExternalInput")
    xk_t = nc.dram_tensor("xk_t", [D, S], BF16, kind="ExternalInput")
    xv_t = nc.dram_tensor("xv_t", [D, S], BF16, kind="ExternalInput")
    m_t = nc.dram_tensor("m_t", [S, QS], BF16, kind="ExternalInput")
    wq_t = nc.dram_tensor("wq_t", [D, D], BF16, kind="ExternalInput")
    wk_t = nc.dram_tensor("wk_t", [D, D], BF16, kind="ExternalInput")
    wv_t = nc.dram_tensor("wv_t", [D, D], BF16, kind="ExternalInput")
    wo_t = nc.dram_tensor("wo_t", [D, D], BF16, kind="ExternalInput")
    bq_c = nc.dram_tensor("bq_c", [128, NKC], F32, kind="ExternalInput")
    bk_c = nc.dram_tensor("bk_c", [128, NKC], F32, kind="ExternalInput")
    bv_r = nc.dram_tensor("bv_r", [1, D], BF16, kind="ExternalInput")
    bo_r = nc.dram_tensor("bo_r", [1, D], BF16, kind="ExternalInput")
    gamma_r = nc.dram_tensor("gamma_r", [1, D], BF16, kind="ExternalInput")
    beta_r = nc.dram_tensor("beta_r", [1, D], BF16, kind="ExternalInput")
    out = nc.dram_tensor("out", [QS, D], F32, kind="ExternalOutput")

    with tile.TileContext(nc) as tc:
        with (
            tc.tile_pool(name="consts", bufs=1) as consts,
            tc.tile_pool(name="kT", bufs=NKC) as kT_pool,
            tc.tile_pool(name="v", bufs=NKT) as v_pool,
            tc.tile_pool(name="qT", bufs=NKC) as qT_pool,
            tc.tile_pool(name="mT", bufs=NKT) as mT_pool,
            tc.tile_pool(name="qnat", bufs=NQT) as qnat_pool,
            tc.tile_pool(name="oT", bufs=NKC) as oT_pool,
            tc.tile_pool(name="wo", bufs=NKC) as wo_pool,
        ):
            # ---- constants / small tensors ----
            ones1_bf = consts.tile([1, 128], BF16)
            nc.vector.memset(ones1_bf, 1.0)
            ones64_f = consts.tile([33, 64], F32)
            nc.vector.memset(ones64_f, 1.0)
            eps_col = consts.tile([128, 1], F32)
            nc.vector.memset(eps_col, EPS)
            bqc_sb = consts.tile([128, NKC], F32)
            nc.sync.dma_start(out=bqc_sb, in_=bq_c[:, :])
            bkc_sb = consts.tile([128, NKC], F32)
            nc.sync.dma_start(out=bkc_sb, in_=bk_c[:, :])
            bvr_sb = consts.tile([1, D], BF16)
            nc.sync.dma_start(out=bvr_sb, in_=bv_r[:, :])
            bor_sb = consts.tile([1, D], BF16)
            nc.sync.dma_start(out=bor_sb, in_=bo_r[:, :])
            gr_sb = consts.tile([1, D], BF16)
            nc.sync.dma_start(out=gr_sb, in_=gamma_r[:, :])
            br_sb = consts.tile([1, D], BF16)
            nc.sync.dma_start(out=br_sb, in_=beta_r[:, :])

            # gamma/beta broadcast to [128, D] via PE outer product
            gb_sb = consts.tile([128, D], F32)
            bb_sb = consts.tile([128, D], F32)
            with tc.tile_pool(name="gb_ps", bufs=2, space="PSUM") as gb_ps_pool:
                for row_sb, dst in ((gr_sb, gb_sb), (br_sb, bb_sb)):
                    for nh in range(2):
                        ps = gb_ps_pool.tile([128, 512], F32, tag="gbps")
                        nc.tensor.matmul(
                            ps, ones1_bf, row_sb[:, nh * 512:(nh + 1) * 512],
                            start=True, stop=True)
                        nc.vector.tensor_copy(
                            out=dst[:, nh * 512:(nh + 1) * 512], in_=ps)

            # ---- phase B: Q projection -> qT_sb (head-transposed [o, tok]) ----
            qT_sb = [qT_pool.tile([128, QS], BF16, tag="qT", name=f"qT{i}")
                     for i in range(NKC)]
            with (
                tc.tile_pool(name="xq", bufs=NKC) as xq_pool,
                tc.tile_pool(name="wq", bufs=NKC) as wq_pool,
                tc.tile_pool(name="psB", bufs=8, space="PSUM") as psB,
            ):
                xq_sb, wq_sb = [], []
                for kc in range(NKC):
                    wt = wq_pool.tile([128, D], BF16, tag="wq", name=f"wq{kc}")
                    nc.sync.dma_start(out=wt, in_=wq_t[kc * 128:(kc + 1) * 128, :])
                    wq_sb.append(wt)
                    xt = xq_pool.tile([128, QS], BF16, tag="xq", name=f"xq{kc}")
                    nc.sync.dma_start(out=xt, in_=xq_t[kc * 128:(kc + 1) * 128, :])
                    xq_sb.append(xt)

                def mmB(ps, ot, kc, start, stop):
                    nc.tensor.matmul(ps, wq_sb[kc][:, ot * 128:(ot + 1) * 128],
                                     xq_sb[kc][:, 0:QS], start=start, stop=stop)

                def finB(ps, ot):
                    nc.vector.tensor_scalar(
                        out=qT_sb[ot], in0=ps, scalar1=bqc_sb[:, ot:ot + 1],
                        scalar2=None, op0=ALU.add)

                _proj_phase(nc, psB, list(range(NKC)), NKC, mmB, finB)

            # ---- phase C: K projection -> kT_sb ----
            kT_sb = [kT_pool.tile([128, S], BF16, tag="kT", name=f"kT{i}")
                     for i in range(NKC)]
            with (
                tc.tile_pool(name="xk", bufs=NKC) as xk_pool,
                tc.tile_pool(name="wk", bufs=NKC) as wk_pool,
                tc.tile_pool(name="psC", bufs=8, space="PSUM") as psC,
            ):
                xk_sb, wk_sb = [], []
                for kc in range(NKC):
                    wt = wk_pool.tile([128, D], BF16, tag="wk", name=f"wk{kc}")
                    nc.sync.dma_start(out=wt, in_=wk_t[kc * 128:(kc + 1) * 128, :])
                    wk_sb.append(wt)
                    xt = xk_pool.tile([128, S], BF16, tag="xk", name=f"xk{kc}")
                    nc.sync.dma_start(out=xt, in_=xk_t[kc * 128:(kc + 1) * 128, :])
                    xk_sb.append(xt)

                def mmC(ps, g, kc, start, stop):
                    ot, tck = g
                    nc.tensor.matmul(ps, wk_sb[kc][:, ot * 128:(ot + 1) * 128],
                                     xk_sb[kc][:, tck * 512:(tck + 1) * 512],
                                     start=start, stop=stop)

                def finC(ps, g):
                    ot, tck = g
                    nc.vector.tensor_scalar(
                        out=kT_sb[ot][:, tck * 512:(tck + 1) * 512],
                        in0=ps, scalar1=bkc_sb[:, ot:ot + 1], scalar2=None,
                        op0=ALU.add)

                groupsC = [(ot, tck) for ot in range(NKC) for tck in range(S // 512)]
                _proj_phase(nc, psC, groupsC, NKC, mmC, finC)

            # ---- phase D: V projection -> v_sb (token-major, ones col at 64) ----
            v_sb = [v_pool.tile([128, H, VW], BF16, tag="v", name=f"v{i}")
                    for i in range(NKT)]
            for kt in range(NKT):
                nc.vector.memset(v_sb[kt][:, :, 64:66], 1.0)
            with (
                tc.tile_pool(name="xv", bufs=NKC) as xv_pool,
                tc.tile_pool(name="wv", bufs=NKC) as wv_pool,
                tc.tile_pool(name="psD", bufs=8, space="PSUM") as psD,
            ):
                xv_sb, wv_sb = [], []
                for kc in range(NKC):
                    wt = wv_pool.tile([128, D], BF16, tag="wv", name=f"wv{kc}")
                    nc.sync.dma_start(out=wt, in_=wv_t[kc * 128:(kc + 1) * 128, :])
                    wv_sb.append(wt)
                    xt = xv_pool.tile([128, S], BF16, tag="xv", name=f"xv{kc}")
                    nc.sync.dma_start(out=xt, in_=xv_t[kc * 128:(kc + 1) * 128, :])
                    xv_sb.append(xt)

                def mmD(ps, g, kc, start, stop):
                    kt, nh = g
                    nc.tensor.matmul(ps, xv_sb[kc][:, kt * 128:(kt + 1) * 128],
                                     wv_sb[kc][:, nh * 512:(nh + 1) * 512],
                                     start=start, stop=False)
                    if stop:
                        nc.tensor.matmul(ps, ones1_bf,
                                         bvr_sb[:, nh * 512:(nh + 1) * 512],
                                         start=False, stop=True)

                def finD(ps, g):
                    kt, nh = g
                    nc.vector.tensor_copy(
                        out=v_sb[kt][:, nh * 8:(nh + 1) * 8, 0:64],
                        in_=ps.rearrange("p (a b) -> p a b", a=8))

                groupsD = [(kt, nh) for kt in range(NKT) for nh in range(2)]
                _proj_phase(nc, psD, groupsD, NKC, mmD, finD)

            # mask tiles + residual + wo prefetch (needed by E/F)
            mT_sb = []
            for kt in range(NKT):
                t = mT_pool.tile([128, QS], BF16, tag="mT", name=f"mT{kt}")
                nc.sync.dma_start(out=t, in_=m_t[kt * 128:(kt + 1) * 128, :])
                mT_sb.append(t)
            qnat_sb = []
            for tt in range(NQT):
                t = qnat_pool.tile([128, D], F32, tag="qnat", name=f"qnat{tt}")
                nc.sync.dma_start(out=t, in_=q_nat[tt * 128:(tt + 1) * 128, :])
                qnat_sb.append(t)
            wo_sb = []
            for kc in range(NKC):
                wt = wo_pool.tile([128, D], BF16, tag="wo", name=f"wo{kc}")
                nc.sync.dma_start(out=wt, in_=wo_t[kc * 128:(kc + 1) * 128, :])
                wo_sb.append(wt)

            # ---- phase E: attention (head pairs; scores transposed [k, q]) ----
            oT_sb = [oT_pool.tile([128, QS], BF16, tag="oT", name=f"oT{i}")
                     for i in range(NKC)]
            with (
                tc.tile_pool(name="psS", bufs=2, space="PSUM") as psS,
                tc.tile_pool(name="psO", bufs=4, space="PSUM") as psO,
                tc.tile_pool(name="pp", bufs=12) as pp_pool,
                tc.tile_pool(name="att_sm", bufs=6) as att_sm,
            ):
                for jj in range(0, H // 2, 2):
                    o_ps = {}
                    for j in (jj, jj + 1):
                        o_ps[j] = [psO.tile([65, QS], F32, tag="psO",
                                            name=f"ops{j}_{i}") for i in range(2)]
                    for kt in range(NKT):
                        for j in (jj, jj + 1):
                            s_ps = psS.tile([128, 2 * QS], F32, tag="psS")
                            for hi in range(2):
                                nc.tensor.matmul(
                                    s_ps[:, hi * QS:(hi + 1) * QS],
                                    kT_sb[j][hi * 64:(hi + 1) * 64,
                                             kt * 128:(kt + 1) * 128],
                                    qT_sb[j][hi * 64:(hi + 1) * 64, 0:QS],
                                    start=True, stop=True,
                                    tile_position=(hi * 64, 0),
                                )
                            p_sb = pp_pool.tile([128, 2 * QS], BF16, tag="pp")
                            nc.scalar.activation(out=p_sb, in_=s_ps, func=AF.Exp)
                            m_rep = bass.AP(
                                tensor=mT_sb[kt].tensor, offset=mT_sb[kt].offset,
                                ap=[mT_sb[kt].ap[0], [0, 2], [1, QS]],
                            )
                            nc.vector.tensor_tensor(
                                out=p_sb.rearrange("p (a b) -> p a b", a=2),
                                in0=p_sb.rearrange("p (a b) -> p a b", a=2),
                                in1=m_rep, op=ALU.mult,
                            )
                            for hi in range(2):
                                nc.tensor.matmul(
                                    o_ps[j][hi],
                                    v_sb[kt][:, 2 * j + hi, 0:65],
                                    p_sb[:, hi * QS:(hi + 1) * QS],
                                    start=(kt == 0), stop=(kt == NKT - 1),
                                )
                    for j in (jj, jj + 1):
                        # r = exp(-ln(d)); one [1,QS] row per head, then
                        # partition-broadcast on the idle GPSIMD engine
                        for hi in range(2):
                            r_row = att_sm.tile([1, QS], F32, tag=f"rrow{hi}",
                                                name=f"rrow{hi}")
                            nc.scalar.activation(out=r_row,
                                                 in_=o_ps[j][hi][64:65, :],
                                                 func=AF.Ln)
                            nc.scalar.activation(out=r_row, in_=r_row,
                                                 func=AF.Exp, scale=-1.0)
                            r_bc = att_sm.tile([64, QS], F32, tag="rbc")
                            nc.gpsimd.partition_broadcast(r_bc, r_row)
                            nc.vector.tensor_tensor(
                                out=oT_sb[j][hi * 64:(hi + 1) * 64, :],
                                in0=o_ps[j][hi][0:64, :], in1=r_bc, op=ALU.mult,
                            )

            # ---- phase F: output projection + residual ----
            with (
                tc.tile_pool(name="psF", bufs=4, space="PSUM") as psF,
            ):
                for tt in range(NQT):
                    for nh in range(2):
                        ps = psF.tile([128, 512], F32, tag="psF")
                        for jc in range(NKC):
                            nc.tensor.matmul(
                                ps, oT_sb[jc][:, tt * 128:(tt + 1) * 128],
                                wo_sb[jc][:, nh * 512:(nh + 1) * 512],
                                start=(jc == 0), stop=False)
                        nc.tensor.matmul(
                            ps, ones1_bf, bor_sb[:, nh * 512:(nh + 1) * 512],
                            start=False, stop=True)
                        nc.vector.tensor_tensor(
                            out=qnat_sb[tt][:, nh * 512:(nh + 1) * 512],
                            in0=ps, in1=qnat_sb[tt][:, nh * 512:(nh + 1) * 512],
                            op=ALU.add)

            # ---- phase G: layernorm + store ----
            with tc.tile_pool(name="ln", bufs=2 * NQT) as ln_pool:
                for tt in range(NQT):
                    y = qnat_sb[tt]
                    stats = ln_pool.tile([128, 2, 6], F32, tag="stats")
                    for half in range(2):
                        nc.vector.bn_stats(
                            out=stats[:, half, :],
                            in_=y[:, half * 512:(half + 1) * 512])
                    mv = ln_pool.tile([128, 2], F32, tag="mv")
                    nc.vector.bn_aggr(out=mv, in_=stats)
                    rstd = ln_pool.tile([128, 1], F32, tag="rstd")
                    nc.scalar.activation(out=rstd, in_=mv[:, 1:2], func=AF.Ln,
                                         bias=eps_col, scale=1.0)
                    nc.scalar.activation(out=rstd, in_=rstd, func=AF.Exp,
                                         scale=-0.5)
                    nc.vector.tensor_scalar(
                        out=y, in0=y, scalar1=mv[:, 0:1], scalar2=rstd,
                        op0=ALU.subtract, op1=ALU.mult)
                    nc.vector.tensor_tensor(out=y, in0=y, in1=gb_sb, op=ALU.mult)
                    nc.vector.tensor_tensor(out=y, in0=y, in1=bb_sb, op=ALU.add)
                    nc.sync.dma_start(out=out[tt * 128:(tt + 1) * 128, :], in_=y)

    nc.compile()
    return nc


_NC_CACHE = None


def _get_nc():
    global _NC_CACHE
    if _NC_CACHE is None:
        _NC_CACHE = build_nc()
    return _NC_CACHE


def _prep_in_maps(query, key, values, mask, Wq, bq, Wk, bk, Wv, bv, Wo, bo,
                  gamma, beta):
    bf = ml_dtypes.bfloat16
    f32 = np.float32

    def c(x):
        return np.ascontiguousarray(x)

    wq_t = c((Wq.T / 8.0).astype(bf))
    wk_t = c(Wk.T.astype(bf))
    wv_t = c(Wv.T.astype(bf))
    wo_t = c(Wo.T.astype(bf))
    bq_c = c((bq.astype(f32) / 8.0).reshape(NKC, 128).T)
    bk_c = c(bk.astype(f32).reshape(NKC, 128).T)
    bv_r = c(bv.astype(bf)[None, :])
    bo_r = c(bo.astype(bf)[None, :])
    gamma_r = c(gamma.astype(bf)[None, :])
    beta_r = c(beta.astype(bf)[None, :])

    xk_t = [c(key[b].T.astype(bf)) for b in range(B)]
    xv_t = [c(values[b].T.astype(bf)) for b in range(B)]
    m_tb = [c(mask[b].T.astype(bf)) for b in range(B)]
    xq_tb = [c(query[b].T.astype(bf)) for b in range(B)]

    in_maps = []
    for core in range(N_CORES):
        b = core // 4
        qs = core % 4
        in_maps.append({
            "xq_t": c(xq_tb[b][:, qs * QS:(qs + 1) * QS]),
            "q_nat": c(query[b, qs * QS:(qs + 1) * QS, :].astype(f32)),
            "xk_t": xk_t[b],
            "xv_t": xv_t[b],
            "m_t": c(m_tb[b][:, qs * QS:(qs + 1) * QS]),
            "wq_t": wq_t, "wk_t": wk_t, "wv_t": wv_t, "wo_t": wo_t,
            "bq_c": bq_c, "bk_c": bk_c, "bv_r": bv_r, "bo_r": bo_r,
            "gamma_r": gamma_r, "beta_r": beta_r,
        })
    return in_maps


def kernel(query, key, values, mask, Wq, bq, Wk, bk, Wv, bv, Wo, bo, gamma,
           beta, _trace=False):
    query = np.asarray(query, dtype=np.float32)
    key = np.asarray(key, dtype=np.float32)
    values = np.asarray(values, dtype=np.float32)
    mask = np.asarray(mask)
    in_maps = _prep_in_maps(query, key, values, mask,
                            np.asarray(Wq, np.float32), np.asarray(bq, np.float32),
                            np.asarray(Wk, np.float32), np.asarray(bk, np.float32),
                            np.asarray(Wv, np.float32), np.asarray(bv, np.float32),
                            np.asarray(Wo, np.float32), np.asarray(bo, np.float32),
                            np.asarray(gamma, np.float32), np.asarray(beta, np.float32))
    nc = _get_nc()
    res = bass_utils.run_bass_kernel_spmd(
        nc, in_maps, core_ids=list(range(N_CORES)), trace=_trace,
    )
    outp = np.empty((B, S, D), dtype=np.float32)
    for core in range(N_CORES):
        b = core // 4
        qs = core % 4
        outp[b, qs * QS:(qs + 1) * QS, :] = res.results[core]["out"]
    if _trace:
        kernel._last_results = res
    return outp
